# revision 19
# baseline (speedup 1.0000x reference)
"""Causal self-attention (B=2, T=2048, C=1024, H=16, D=64) on 8 trn2 cores.

Sharding: core c handles batch c//4 and heads 4*(c%4)..4*(c%4)+3.  Each
core computes its heads' QKV projection, causal attention, and the
partial output projection (W_proj row-shard); the 4 partials per batch
are summed on the host (the Megatron all-reduce done at gather time).

On-core dataflow is feature-major throughout:
  X^T arrives pre-transposed and window-packed from the host (f16).
  Q^T,K^T = W^T X^T ; V natural = (X^T chunk)^T Wv
  S^T[k,q] = K Q^T per 128-k-chunk (block-causal)
  P^T = exp(S^T/8) (ACT), diagonal blocks masked (GpSimd)
  O'^T += V'^T P^T where V' carries ones columns so PSUM rows 64:128
  accumulate the softmax denominator; O^T = O'^T[0:64] * recip(rows 64+).
  Y^T = W_proj^T O^T -> f16 [1024, 2048] partial per core.

PE discipline: every matmul in the kernel is a full (128,128)-tile
matmul -- the PE never changes tiling mode, so it never drains.  The
64-deep S contraction is padded to 128 with zeros held in per-head K^T
tiles (kt_z[h]: head h's K on its own 64 partitions, zeros on the other
64, so the packed Q tile can be used as the moving operand directly).
Next-round QKV and prev-round proj are spliced between attention groups
as PE filler while ACT streams exps; PV lags its exp by one group.
b_k is dropped (softmax shift invariance), b_v folds into the host-side
bias, b_q rides the Q staging add.  ACT: exps only.  GpSimd: causal
masks + some DMA issue.  DVE: all PSUM evacuation.  Host pre-arranges
x/w layouts so every DMA is wide-row contiguous and cheap to issue.
"""
import os
import sys
import numpy as np

B, T, C = 2, 2048, 1024
H, D = 16, 64
HPC = 4                 # heads per core
QC = HPC * D            # 256 qkv cols per core
NCORES = 8
NT = T // 128           # 16 k-chunks of 128
NT4 = T // 512          # 4 q-window rounds of 512
NKC = 8                 # contraction chunks over C
SCALE = 1.0 / np.sqrt(D)

_cache = {}


def _ensure_env():
    for p in ("/opt/trn_rl_repo", "/root/.axon_site/_ro/trn_rl_repo"):
        if os.path.isdir(p) and p not in sys.path:
            sys.path.append(p)
    jp = os.environ.get("JAX_PLATFORMS")
    if jp and "axon" not in jp and "jax" not in sys.modules:
        os.environ["JAX_PLATFORMS"] = ""


def _groups_for(t4):
    """Pack the causal kc-chunks of q-window t4 into <=1024-col PSUM
    tiles.  Each group is a list of (kc, lo, n, off): k-chunk index,
    absolute q start, cols, col offset in the PSUM tile.  512-alignment
    keeps every matmul inside one PSUM bank row."""
    lo0, hi0 = t4 * 512, (t4 + 1) * 512
    last_kc = 4 * t4 + 3
    groups, cur, pos = [], [], 0
    for kc in range(last_kc + 1):
        lo = max(lo0, kc * 128)
        n = hi0 - lo
        npos = pos if pos % 512 + n <= 512 else (pos + 511) // 512 * 512
        if npos + n > 1024:
            groups.append(cur)
            cur, npos = [], 0
        cur.append((kc, lo, n, npos))
        pos = npos + n
    groups.append(cur)
    return groups


def _build():
    import concourse.bass as bass
    import concourse.bacc as bacc
    import concourse.mybir as mybir
    import concourse.tile as tile

    F32 = mybir.dt.float32
    F16 = mybir.dt.float16
    AF = mybir.ActivationFunctionType
    MUL = bass.mybir.AluOpType.mult

    nc = bacc.Bacc()
    # x^T window-packed: [128, t4*(c*512)] so each q-window is one
    # contiguous wide-row DMA
    xt_d = nc.dram_tensor("xt", [128, NT4 * NKC * 512], F16,
                          kind="ExternalInput")
    # weights pre-arranged to [128, c*cols] on the host
    wq_d = nc.dram_tensor("wq", [128, NKC * QC], F16, kind="ExternalInput")
    wk_d = nc.dram_tensor("wk", [128, NKC * QC], F16, kind="ExternalInput")
    wv_d = nc.dram_tensor("wv", [128, NKC * QC], F16, kind="ExternalInput")
    bq_d = nc.dram_tensor("bq", [128, 2], F32, kind="ExternalInput")
    wp_d = nc.dram_tensor("wp", [QC, C], F16, kind="ExternalInput")
    mask_d = nc.dram_tensor("mask", [128, 128], F16, kind="ExternalInput")
    yt_d = nc.dram_tensor("yt", [C, T], F16, kind="ExternalOutput")

    with tile.TileContext(nc) as tc:
        with tc.tile_pool(name="cst", bufs=1) as cst, \
             tc.tile_pool(name="wgt", bufs=1) as wgt, \
             tc.tile_pool(name="xt", bufs=4) as xtp, \
             tc.tile_pool(name="qk", bufs=1) as qkp, \
             tc.tile_pool(name="vv", bufs=1) as vvp, \
             tc.tile_pool(name="pp", bufs=8) as ppp, \
             tc.tile_pool(name="dn", bufs=1) as dnp, \
             tc.tile_pool(name="yy", bufs=4) as yyp, \
             tc.tile_pool(name="mm", bufs=2, space="PSUM") as mmp, \
             tc.tile_pool(name="ss", bufs=2, space="PSUM") as ssp, \
             tc.tile_pool(name="po", bufs=2, space="PSUM") as pop:

            # ---- weights / x first: they gate the first matmuls.
            # Split every gating transfer across several dma_starts on
            # different sequencers: a single dma_start only sustains
            # ~60 GB/s, parallel ones stack.
            wq_s = wgt.tile([128, NKC * QC], F16, tag="wq")
            wk_s = wgt.tile([128, NKC * QC], F16, tag="wk")
            wv_s = wgt.tile([128, NKC * QC], F16, tag="wv")

            def split_dma(engs, dst, src, nsplit):
                tot = dst.shape[1]
                step = tot // nsplit
                for i in range(nsplit):
                    engs[i % len(engs)].dma_start(
                        out=dst[:, i * step:(i + 1) * step],
                        in_=src[:, i * step:(i + 1) * step])

            xts_all = [None] * NT4

            def load_xts(t4, engs, nsplit=2):
                xs = xtp.tile([128, NKC * 512], F16, tag="xt")
                split_dma(engs, xs[:],
                          xt_d[:, t4 * NKC * 512:(t4 + 1) * NKC * 512],
                          nsplit)
                xts_all[t4] = xs

            split_dma([nc.scalar, nc.sync, nc.gpsimd], wq_s[:], wq_d[:], 4)
            load_xts(0, [nc.scalar, nc.sync, nc.gpsimd], 8)
            split_dma([nc.scalar, nc.sync], wk_s[:], wk_d[:], 2)
            split_dma([nc.scalar, nc.sync], wv_s[:], wv_d[:], 2)
            load_xts(1, [nc.sync, nc.gpsimd], 2)

            # dummy matmuls on a scratch tile: keep the PE busy through
            # the initial DMA wait so HAM is at full clock (K=8/8) when
            # the real stream starts
            scratch = cst.tile([128, 512], F16, tag="scr")
            nc.vector.memset(scratch[:], 0.0)
            for i in range(34):
                pd = mmp.tile([128, 512], F32, tag="mm")
                nc.tensor.matmul(pd[:], scratch[:, 0:128], scratch[:],
                                 start=True, stop=True)

            mask = cst.tile([128, 128], F16, tag="mask")
            nc.gpsimd.dma_start(out=mask[:], in_=mask_d[:])
            bq_s = cst.tile([128, 2], F32, tag="bq")
            nc.gpsimd.dma_start(out=bq_s[:], in_=bq_d[:])
            wp_s = [wgt.tile([128, C], F16, tag=f"wp{k}", name=f"wp{k}")
                    for k in range(2)]
            for k in range(2):
                nc.gpsimd.dma_start(out=wp_s[k][:],
                                    in_=wp_d[k * 128:(k + 1) * 128, :])

            # ---- persistent activations ----
            # Q^T / O^T: heads (2m, 2m+1) stacked on partitions
            qt_s = [qkp.tile([128, T], F16, tag=f"qt{m}", name=f"qt{m}")
                    for m in range(2)]
            ot_s = [qkp.tile([128, T], F16, tag=f"ot{m}", name=f"ot{m}")
                    for m in range(2)]
            # K^T per head, zero-padded on the other head's partitions:
            # head h occupies partitions (h%2)*64..+64, the rest stays 0
            # (loaded once from DRAM zeros).  This keeps every S matmul a
            # full 128-contraction (128,128)-tile op -- no PE mode
            # switches anywhere in the kernel -- while the packed Q tile
            # serves as the moving operand unchanged.
            kt_z = [qkp.tile([128, T], F16, tag=f"kt{h}", name=f"kt{h}")
                    for h in range(HPC)]
            for h in range(HPC):
                z0 = 64 if h % 2 == 0 else 0
                nc.vector.memset(kt_z[h][z0:z0 + 64, :], 0.0)
            # V': one tile, [128, kc*(4 heads)*(64 v | 64 ones)]; the
            # ones columns make PSUM rows 64:128 of PV the denominator.
            vp_s = vvp.tile([128, NT * HPC * 2 * D], F16, tag="vp")
            nc.gpsimd.memset(
                vp_s[:].rearrange("p (k h e) -> p k h e",
                                  k=NT, e=2 * D)[:, :, :, D:2 * D], 1.0)

            def vp_slice(kc, h):
                base = (kc * HPC + h) * 2 * D
                return vp_s[:, base:base + 2 * D]

            def unit_q(t4, m):
                """One Q^T projection psum-group (heads 2m, 2m+1)."""
                xts = xts_all[t4]
                pq = mmp.tile([128, 512], F32, tag="mm")
                for c in range(NKC):
                    nc.tensor.matmul(
                        pq[:],
                        wq_s[:, c * QC + m * 128:c * QC + (m + 1) * 128],
                        xts[:, c * 512:(c + 1) * 512],
                        start=(c == 0), stop=(c == NKC - 1))
                nc.vector.tensor_scalar_add(
                    qt_s[m][:, t4 * 512:(t4 + 1) * 512], pq[:],
                    bq_s[:, m:m + 1])

            def unit_k(t4, m):
                """One K^T projection psum-group, evacuated into the two
                per-head zero-padded tiles (partition-aligned copies)."""
                xts = xts_all[t4]
                pk = mmp.tile([128, 512], F32, tag="mm")
                for c in range(NKC):
                    nc.tensor.matmul(
                        pk[:],
                        wk_s[:, c * QC + m * 128:c * QC + (m + 1) * 128],
                        xts[:, c * 512:(c + 1) * 512],
                        start=(c == 0), stop=(c == NKC - 1))
                w0 = t4 * 512
                nc.vector.tensor_copy(
                    kt_z[2 * m][0:64, w0:w0 + 512], pk[0:64, :])
                nc.vector.tensor_copy(
                    kt_z[2 * m + 1][64:128, w0:w0 + 512], pk[64:128, :])

            def unit_v(t4, i2):
                """V for k-chunk pair (4*t4+2*i2, +1), packed in one
                psum tile and evacuated with a single DVE copy."""
                xts = xts_all[t4]
                pv = mmp.tile([128, 512], F32, tag="mm")
                for j in range(2):
                    i = 2 * i2 + j
                    for c in range(NKC):
                        nc.tensor.matmul(
                            pv[:, j * QC:(j + 1) * QC],
                            xts[:, c * 512 + i * 128:c * 512 + (i + 1) * 128],
                            wv_s[:, c * QC:(c + 1) * QC],
                            start=(c == 0), stop=(c == NKC - 1))
                kc = 4 * t4 + 2 * i2
                dst = vp_s[:, kc * QC * 2:(kc + 2) * QC * 2]
                nc.vector.tensor_copy(
                    dst.rearrange("p (k h e) -> p k h e",
                                  k=2, e=2 * D)[:, :, :, 0:D],
                    pv[:].rearrange("p (k h d) -> p k h d", k=2, d=D))

            yt_engs = [nc.sync, nc.gpsimd]

            def unit_proj(t4, m):
                """One output-projection m-chunk for q-window t4."""
                lo0 = t4 * 512
                py = mmp.tile([128, 512], F32, tag="mm")
                for k in range(2):
                    nc.tensor.matmul(py[:],
                                     wp_s[k][:, m * 128:(m + 1) * 128],
                                     ot_s[k][:, lo0:lo0 + 512],
                                     start=(k == 0), stop=(k == 1))
                yt_stage = yyp.tile([128, 512], F16, tag="yt")
                if t4 == NT4 - 1 and m % 2 == 1:
                    # final round drains in the tail: share the
                    # evacuation with the (by now idle) ACT engine
                    nc.scalar.activation(yt_stage[:], py[:], AF.Copy)
                else:
                    nc.vector.tensor_copy(yt_stage[:], py[:])
                if t4 == NT4 - 1:
                    for half in range(2):
                        yt_engs[(m + half) % 2].dma_start(
                            out=yt_d[m * 128:(m + 1) * 128,
                                     lo0 + half * 256:lo0 + (half + 1) * 256],
                            in_=yt_stage[:, half * 256:(half + 1) * 256])
                else:
                    yt_engs[m % 2].dma_start(
                        out=yt_d[m * 128:(m + 1) * 128, lo0:lo0 + 512],
                        in_=yt_stage[:])

            def qkv_units(t4):
                return ([(lambda t=t4, m=m: unit_q(t, m)) for m in range(2)] +
                        [(lambda t=t4, m=m: unit_k(t, m)) for m in range(2)] +
                        [(lambda t=t4, i=i: unit_v(t, i)) for i in range(2)])

            def proj_units(t4):
                return [(lambda t=t4, m=m: unit_proj(t, m)) for m in range(8)]

            # ---- main pipeline ----
            for u in qkv_units(0):
                u()
            for t4 in range(NT4):
                groups = _groups_for(t4)
                if t4 + 2 < NT4:
                    load_xts(t4 + 2, [nc.sync, nc.scalar], 2)
                fillers = (qkv_units(t4 + 1) if t4 + 1 < NT4 else [])
                if t4 == 2:
                    fillers += proj_units(0)
                elif t4 == 3:
                    fillers += proj_units(1) + proj_units(2)
                fillers = fillers[::-1]  # pop() takes from the front
                nslots = HPC * len(groups)
                take = [False] * nslots
                reserve = 3 if t4 == NT4 - 1 else 0
                nf = min(max(len(fillers) - reserve, 0), nslots)
                for j in range(nf):
                    if t4 == NT4 - 1:
                        take[nslots - 1 - (j * nslots) // nf] = True
                    else:
                        take[(j * nslots) // nf] = True
                slot = 0
                lo0, hi0 = t4 * 512, (t4 + 1) * 512
                last_kc = 4 * t4 + 3

                def pv_group(op_tl, h, pt, grp):
                    for (kc, lo, n, off) in grp:
                        nc.tensor.matmul(
                            op_tl[:, lo - lo0:512],
                            vp_slice(kc, h), pt[:, off:off + n],
                            start=(kc == 0), stop=(kc == last_kc))

                def norm_head(h, op_tl):
                    # normalize: PSUM rows 64:128 hold the denominator
                    rc_in = dnp.tile([64, 512], F32, tag="rci", bufs=3)
                    nc.vector.tensor_copy(rc_in[:], op_tl[D:2 * D, :])
                    rc = dnp.tile([64, 512], F32, tag="rc", bufs=3)
                    nc.vector.reciprocal_approx_fast(rc[:], rc_in[:])
                    nc.vector.tensor_tensor(
                        ot_s[h // 2][(h % 2) * 64:(h % 2) * 64 + 64,
                                     lo0:hi0],
                        op_tl[0:D, :], rc[:], op=MUL)

                # PV lags its exp by two groups (queue crosses head
                # boundaries) so the PE never reaches a PV before its
                # exp+mask are long done.
                pending = []

                def pop_pending():
                    op_tl, h, pt, grp, last = pending.pop(0)
                    pv_group(op_tl, h, pt, grp)
                    if last:
                        norm_head(h, op_tl)

                for h in range(HPC):
                    qt_h = qt_s[h // 2]          # packed, full 128 rows
                    kt_h = kt_z[h]               # zero-padded stationary
                    op_tl = pop.tile([128, 512], F32, tag="po")
                    for gi, grp in enumerate(groups):
                        sp = ssp.tile([128, 1024], F32, tag="ss")
                        for (kc, lo, n, off) in grp:
                            nc.tensor.matmul(
                                sp[:, off:off + n],
                                kt_h[:, kc * 128:kc * 128 + 128],
                                qt_h[:, lo:hi0], start=True, stop=True)
                        end = grp[-1][3] + grp[-1][2]
                        pt = ppp.tile([128, 1024], F16, tag="p")
                        nc.scalar.activation(pt[:, 0:end], sp[:, 0:end],
                                             AF.Exp, scale=float(SCALE))
                        for (kc, lo, n, off) in grp:
                            if kc * 128 >= lo0:  # diagonal block
                                nc.gpsimd.tensor_tensor(
                                    pt[:, off:off + 128],
                                    pt[:, off:off + 128], mask[:], op=MUL)
                        pending.append((op_tl, h, pt, grp,
                                        gi == len(groups) - 1))
                        if take[slot] and fillers:
                            fillers.pop()()
                        slot += 1
                        while len(pending) > 2:
                            pop_pending()
                while pending:
                    pop_pending()
                    if fillers:
                        fillers.pop()()
                while fillers:
                    fillers.pop()()
            for u in proj_units(NT4 - 1):
                u()

    nc.finalize()
    return nc


def _get_program():
    if "nc" not in _cache:
        _ensure_env()
        _cache["nc"] = _build()
    return _cache["nc"]


def kernel(x, w_attn, b_attn, w_proj, b_proj):
    x = np.asarray(x, dtype=np.float32)
    w_attn = np.asarray(w_attn, dtype=np.float32)
    b_attn = np.asarray(b_attn, dtype=np.float32)
    w_proj = np.asarray(w_proj, dtype=np.float32)
    b_proj = np.asarray(b_proj, dtype=np.float32)

    nc = _get_program()
    from concourse.bass_utils import run_bass_kernel_spmd

    mask = np.triu(np.ones((128, 128), dtype=np.float16))

    def arrange_w(w):
        # [C, QC] -> [128, NKC*QC]: row p, col c*QC+n = w[c*128+p, n]
        return np.ascontiguousarray(
            w.reshape(NKC, 128, QC).transpose(1, 0, 2).reshape(128, -1)
            .astype(np.float16))

    in_maps = []
    for c in range(NCORES):
        b = c // 4
        hg = c % 4
        q0 = hg * QC
        # x^T window-packed: [128, (t4, c, t)] with
        # value = x[b][t4*512+t, c*128+p]
        xtw = np.ascontiguousarray(
            x[b].astype(np.float16).T            # [C, T]
            .reshape(NKC, 128, NT4, 512)         # [c, p, t4, t]
            .transpose(1, 2, 0, 3)               # [p, t4, c, t]
            .reshape(128, -1))
        in_maps.append({
            "xt": xtw,
            "wq": arrange_w(w_attn[:, q0:q0 + QC]),
            "wk": arrange_w(w_attn[:, C + q0:C + q0 + QC]),
            "wv": arrange_w(w_attn[:, 2 * C + q0:2 * C + q0 + QC]),
            "bq": np.ascontiguousarray(
                b_attn[q0:q0 + QC].reshape(2, 128).T),
            "wp": np.ascontiguousarray(
                w_proj[q0:q0 + QC, :].astype(np.float16)),
            "mask": mask,
        })

    trace = bool(os.environ.get("KERNEL_TRACE"))
    res = run_bass_kernel_spmd(nc, in_maps, list(range(NCORES)), trace=trace)
    _cache["last_results"] = res

    # b_k cancels in the softmax; b_v contributes bv @ W_proj, constant
    # over t, so it joins b_proj at gather time.
    bias = b_proj + b_attn[2 * C:] @ w_proj
    out = np.empty((B, T, C), dtype=np.float32)
    for b in range(B):
        acc = res.results[4 * b]["yt"].astype(np.float32)
        for c in range(4 * b + 1, 4 * b + 4):
            acc = acc + res.results[c]["yt"].astype(np.float32)
        out[b] = acc.T + bias
    return out


# revision 20
# speedup vs baseline: 1.0328x; 1.0328x over previous
"""Causal self-attention (B=2, T=2048, C=1024, H=16, D=64) on 8 trn2 cores.

Sharding: core c handles batch c//4 and heads 4*(c%4)..4*(c%4)+3.  Each
core computes its heads' QKV projection, causal attention, and the
partial output projection (W_proj row-shard); the 4 partials per batch
are summed on the host (the Megatron all-reduce done at gather time).

On-core dataflow is feature-major throughout:
  X^T arrives pre-transposed and window-packed from the host (f16).
  Q^T,K^T = W^T X^T ; V natural = (X^T chunk)^T Wv
  S^T[k,q] = K Q^T per 128-k-chunk (block-causal)
  P^T = exp(S^T/8) (ACT), diagonal blocks masked (GpSimd)
  O'^T += V'^T P^T where V' carries ones columns so PSUM rows 64:128
  accumulate the softmax denominator; O^T = O'^T[0:64] * recip(rows 64+).
  Y^T = W_proj^T O^T -> f16 [1024, 2048] partial per core.

PE discipline: every matmul in the kernel is a full (128,128)-tile
matmul -- the PE never changes tiling mode, so it never drains.  The
64-deep S contraction is padded to 128 with zeros held in per-head K^T
tiles (kt_z[h]: head h's K on its own 64 partitions, zeros on the other
64, so the packed Q tile can be used as the moving operand directly).
Next-round QKV and prev-round proj are spliced between attention groups
as PE filler while ACT streams exps; PV lags its exp by one group.
b_k is dropped (softmax shift invariance), b_v folds into the host-side
bias, b_q rides the Q staging add.  ACT: exps only.  GpSimd: causal
masks + some DMA issue.  DVE: all PSUM evacuation.  Host pre-arranges
x/w layouts so every DMA is wide-row contiguous and cheap to issue.
"""
import os
import sys
import numpy as np

B, T, C = 2, 2048, 1024
H, D = 16, 64
HPC = 4                 # heads per core
QC = HPC * D            # 256 qkv cols per core
NCORES = 8
NT = T // 128           # 16 k-chunks of 128
NT4 = T // 512          # 4 q-window rounds of 512
NKC = 8                 # contraction chunks over C
SCALE = 1.0 / np.sqrt(D)

_cache = {}


def _ensure_env():
    for p in ("/opt/trn_rl_repo", "/root/.axon_site/_ro/trn_rl_repo"):
        if os.path.isdir(p) and p not in sys.path:
            sys.path.append(p)
    jp = os.environ.get("JAX_PLATFORMS")
    if jp and "axon" not in jp and "jax" not in sys.modules:
        os.environ["JAX_PLATFORMS"] = ""


def _groups_for(t4):
    """Pack the causal kc-chunks of q-window t4 into <=1024-col PSUM
    tiles.  Each group is a list of (kc, lo, n, off): k-chunk index,
    absolute q start, cols, col offset in the PSUM tile.  512-alignment
    keeps every matmul inside one PSUM bank row."""
    lo0, hi0 = t4 * 512, (t4 + 1) * 512
    last_kc = 4 * t4 + 3
    groups, cur, pos = [], [], 0
    for kc in range(last_kc + 1):
        lo = max(lo0, kc * 128)
        n = hi0 - lo
        npos = pos if pos % 512 + n <= 512 else (pos + 511) // 512 * 512
        if npos + n > 1024:
            groups.append(cur)
            cur, npos = [], 0
        cur.append((kc, lo, n, npos))
        pos = npos + n
    groups.append(cur)
    return groups


def _build():
    import concourse.bass as bass
    import concourse.bacc as bacc
    import concourse.mybir as mybir
    import concourse.tile as tile

    F32 = mybir.dt.float32
    F16 = mybir.dt.float16
    AF = mybir.ActivationFunctionType
    MUL = bass.mybir.AluOpType.mult

    nc = bacc.Bacc()
    # x^T window-packed: [128, t4*(c*512)] so each q-window is one
    # contiguous wide-row DMA
    xt_d = nc.dram_tensor("xt", [128, NT4 * NKC * 512], F16,
                          kind="ExternalInput")
    # weights pre-arranged to [128, c*cols] on the host
    wq_d = nc.dram_tensor("wq", [128, NKC * QC], F16, kind="ExternalInput")
    wk_d = nc.dram_tensor("wk", [128, NKC * QC], F16, kind="ExternalInput")
    wv_d = nc.dram_tensor("wv", [128, NKC * QC], F16, kind="ExternalInput")
    bq_d = nc.dram_tensor("bq", [128, 2], F32, kind="ExternalInput")
    wp_d = nc.dram_tensor("wp", [QC, C], F16, kind="ExternalInput")
    mask_d = nc.dram_tensor("mask", [128, 128], F16, kind="ExternalInput")
    yt_d = nc.dram_tensor("yt", [C, T], F16, kind="ExternalOutput")

    with tile.TileContext(nc) as tc:
        with tc.tile_pool(name="cst", bufs=1) as cst, \
             tc.tile_pool(name="wgt", bufs=1) as wgt, \
             tc.tile_pool(name="xt", bufs=4) as xtp, \
             tc.tile_pool(name="qk", bufs=1) as qkp, \
             tc.tile_pool(name="vv", bufs=1) as vvp, \
             tc.tile_pool(name="pp", bufs=8) as ppp, \
             tc.tile_pool(name="dn", bufs=1) as dnp, \
             tc.tile_pool(name="yy", bufs=4) as yyp, \
             tc.tile_pool(name="mm", bufs=2, space="PSUM") as mmp, \
             tc.tile_pool(name="ss", bufs=2, space="PSUM") as ssp, \
             tc.tile_pool(name="po", bufs=2, space="PSUM") as pop:

            # ---- weights / x first: they gate the first matmuls.
            # Split every gating transfer across several dma_starts on
            # different sequencers: a single dma_start only sustains
            # ~60 GB/s, parallel ones stack.
            wq_s = wgt.tile([128, NKC * QC], F16, tag="wq")
            wk_s = wgt.tile([128, NKC * QC], F16, tag="wk")
            wv_s = wgt.tile([128, NKC * QC], F16, tag="wv")

            def split_dma(engs, dst, src, nsplit):
                tot = dst.shape[1]
                step = tot // nsplit
                for i in range(nsplit):
                    engs[i % len(engs)].dma_start(
                        out=dst[:, i * step:(i + 1) * step],
                        in_=src[:, i * step:(i + 1) * step])

            xts_all = [None] * NT4

            def load_xts(t4, engs, nsplit=2):
                xs = xtp.tile([128, NKC * 512], F16, tag="xt")
                split_dma(engs, xs[:],
                          xt_d[:, t4 * NKC * 512:(t4 + 1) * NKC * 512],
                          nsplit)
                xts_all[t4] = xs

            split_dma([nc.scalar, nc.sync, nc.gpsimd], wq_s[:], wq_d[:], 4)
            load_xts(0, [nc.scalar, nc.sync, nc.gpsimd], 8)
            split_dma([nc.scalar, nc.sync], wk_s[:], wk_d[:], 2)
            split_dma([nc.scalar, nc.sync], wv_s[:], wv_d[:], 2)
            load_xts(1, [nc.sync, nc.gpsimd], 2)

            # dummy matmuls on a scratch tile: keep the PE busy through
            # the initial DMA wait so HAM is at full clock (K=8/8) when
            # the real stream starts
            scratch = cst.tile([128, 512], F16, tag="scr")
            nc.vector.memset(scratch[:], 0.0)
            for i in range(34):
                pd = mmp.tile([128, 512], F32, tag="mm")
                nc.tensor.matmul(pd[:], scratch[:, 0:128], scratch[:],
                                 start=True, stop=True)

            mask = cst.tile([128, 128], F16, tag="mask")
            nc.gpsimd.dma_start(out=mask[:], in_=mask_d[:])
            bq_s = cst.tile([128, 2], F32, tag="bq")
            nc.gpsimd.dma_start(out=bq_s[:], in_=bq_d[:])
            wp_s = [wgt.tile([128, C], F16, tag=f"wp{k}", name=f"wp{k}")
                    for k in range(2)]
            for k in range(2):
                nc.gpsimd.dma_start(out=wp_s[k][:],
                                    in_=wp_d[k * 128:(k + 1) * 128, :])

            # ---- persistent activations ----
            # Q^T / O^T: heads (2m, 2m+1) stacked on partitions
            qt_s = [qkp.tile([128, T], F16, tag=f"qt{m}", name=f"qt{m}")
                    for m in range(2)]
            ot_s = [qkp.tile([128, T], F16, tag=f"ot{m}", name=f"ot{m}")
                    for m in range(2)]
            # K^T per head, zero-padded on the other head's partitions:
            # head h occupies partitions (h%2)*64..+64, the rest stays 0
            # (loaded once from DRAM zeros).  This keeps every S matmul a
            # full 128-contraction (128,128)-tile op -- no PE mode
            # switches anywhere in the kernel -- while the packed Q tile
            # serves as the moving operand unchanged.
            kt_z = [qkp.tile([128, T], F16, tag=f"kt{h}", name=f"kt{h}")
                    for h in range(HPC)]
            for h in range(HPC):
                z0 = 64 if h % 2 == 0 else 0
                nc.vector.memset(kt_z[h][z0:z0 + 64, :], 0.0)
            # V': one tile, [128, kc*(4 heads)*(64 v | 64 ones)]; the
            # ones columns make PSUM rows 64:128 of PV the denominator.
            vp_s = vvp.tile([128, NT * HPC * 2 * D], F16, tag="vp")
            nc.gpsimd.memset(
                vp_s[:].rearrange("p (k h e) -> p k h e",
                                  k=NT, e=2 * D)[:, :, :, D:2 * D], 1.0)

            def vp_slice(kc, h):
                base = (kc * HPC + h) * 2 * D
                return vp_s[:, base:base + 2 * D]

            def unit_q(t4, m):
                """One Q^T projection psum-group (heads 2m, 2m+1)."""
                xts = xts_all[t4]
                pq = mmp.tile([128, 512], F32, tag="mm")
                for c in range(NKC):
                    nc.tensor.matmul(
                        pq[:],
                        wq_s[:, c * QC + m * 128:c * QC + (m + 1) * 128],
                        xts[:, c * 512:(c + 1) * 512],
                        start=(c == 0), stop=(c == NKC - 1))
                nc.vector.tensor_scalar_add(
                    qt_s[m][:, t4 * 512:(t4 + 1) * 512], pq[:],
                    bq_s[:, m:m + 1])

            def unit_k(t4, m):
                """One K^T projection psum-group, evacuated into the two
                per-head zero-padded tiles (partition-aligned copies)."""
                xts = xts_all[t4]
                pk = mmp.tile([128, 512], F32, tag="mm")
                for c in range(NKC):
                    nc.tensor.matmul(
                        pk[:],
                        wk_s[:, c * QC + m * 128:c * QC + (m + 1) * 128],
                        xts[:, c * 512:(c + 1) * 512],
                        start=(c == 0), stop=(c == NKC - 1))
                w0 = t4 * 512
                nc.vector.tensor_copy(
                    kt_z[2 * m][0:64, w0:w0 + 512], pk[0:64, :])
                nc.vector.tensor_copy(
                    kt_z[2 * m + 1][64:128, w0:w0 + 512], pk[64:128, :])

            def unit_v(t4, i2):
                """V for k-chunk pair (4*t4+2*i2, +1), packed in one
                psum tile and evacuated with a single DVE copy."""
                xts = xts_all[t4]
                pv = mmp.tile([128, 512], F32, tag="mm")
                for j in range(2):
                    i = 2 * i2 + j
                    for c in range(NKC):
                        nc.tensor.matmul(
                            pv[:, j * QC:(j + 1) * QC],
                            xts[:, c * 512 + i * 128:c * 512 + (i + 1) * 128],
                            wv_s[:, c * QC:(c + 1) * QC],
                            start=(c == 0), stop=(c == NKC - 1))
                kc = 4 * t4 + 2 * i2
                dst = vp_s[:, kc * QC * 2:(kc + 2) * QC * 2]
                nc.vector.tensor_copy(
                    dst.rearrange("p (k h e) -> p k h e",
                                  k=2, e=2 * D)[:, :, :, 0:D],
                    pv[:].rearrange("p (k h d) -> p k h d", k=2, d=D))

            yt_engs = [nc.sync, nc.gpsimd]

            def unit_proj(t4, m):
                """One output-projection m-chunk for q-window t4."""
                lo0 = t4 * 512
                py = mmp.tile([128, 512], F32, tag="mm")
                for k in range(2):
                    nc.tensor.matmul(py[:],
                                     wp_s[k][:, m * 128:(m + 1) * 128],
                                     ot_s[k][:, lo0:lo0 + 512],
                                     start=(k == 0), stop=(k == 1))
                yt_stage = yyp.tile([128, 512], F16, tag="yt")
                if t4 == NT4 - 1 and m % 2 == 1:
                    # final round drains in the tail: share the
                    # evacuation with the (by now idle) ACT engine
                    nc.scalar.activation(yt_stage[:], py[:], AF.Copy)
                else:
                    nc.vector.tensor_copy(yt_stage[:], py[:])
                if t4 == NT4 - 1:
                    for half in range(2):
                        yt_engs[(m + half) % 2].dma_start(
                            out=yt_d[m * 128:(m + 1) * 128,
                                     lo0 + half * 256:lo0 + (half + 1) * 256],
                            in_=yt_stage[:, half * 256:(half + 1) * 256])
                else:
                    yt_engs[m % 2].dma_start(
                        out=yt_d[m * 128:(m + 1) * 128, lo0:lo0 + 512],
                        in_=yt_stage[:])

            def qkv_units(t4):
                return ([(lambda t=t4, m=m: unit_q(t, m)) for m in range(2)] +
                        [(lambda t=t4, m=m: unit_k(t, m)) for m in range(2)] +
                        [(lambda t=t4, i=i: unit_v(t, i)) for i in range(2)])

            def proj_units(t4):
                return [(lambda t=t4, m=m: unit_proj(t, m)) for m in range(8)]

            # ---- main pipeline ----
            for u in qkv_units(0):
                u()
            for t4 in range(NT4):
                groups = _groups_for(t4)
                if t4 + 2 < NT4:
                    load_xts(t4 + 2, [nc.sync, nc.scalar], 2)
                fillers = (qkv_units(t4 + 1) if t4 + 1 < NT4 else [])
                if t4 == 2:
                    fillers += proj_units(0)
                elif t4 == 3:
                    fillers += proj_units(1) + proj_units(2)
                fillers = fillers[::-1]  # pop() takes from the front
                nslots = HPC * len(groups)
                take = [False] * nslots
                reserve = 3 if t4 >= NT4 - 2 else 0
                nf = min(max(len(fillers) - reserve, 0), nslots)
                for j in range(nf):
                    if t4 == NT4 - 1:
                        take[nslots - 1 - (j * nslots) // nf] = True
                    else:
                        take[(j * nslots) // nf] = True
                slot = 0
                lo0, hi0 = t4 * 512, (t4 + 1) * 512
                last_kc = 4 * t4 + 3

                def pv_group(op_tl, h, pt, grp):
                    for (kc, lo, n, off) in grp:
                        nc.tensor.matmul(
                            op_tl[:, lo - lo0:512],
                            vp_slice(kc, h), pt[:, off:off + n],
                            start=(kc == 0), stop=(kc == last_kc))

                def norm_head(h, op_tl):
                    # normalize: PSUM rows 64:128 hold the denominator
                    rc_in = dnp.tile([64, 512], F32, tag="rci", bufs=3)
                    nc.vector.tensor_copy(rc_in[:], op_tl[D:2 * D, :])
                    rc = dnp.tile([64, 512], F32, tag="rc", bufs=3)
                    nc.vector.reciprocal_approx_fast(rc[:], rc_in[:])
                    nc.vector.tensor_tensor(
                        ot_s[h // 2][(h % 2) * 64:(h % 2) * 64 + 64,
                                     lo0:hi0],
                        op_tl[0:D, :], rc[:], op=MUL)

                # PV lags its exp by two groups (queue crosses head
                # boundaries) so the PE never reaches a PV before its
                # exp+mask are long done.
                pending = []

                def pop_pending():
                    op_tl, h, pt, grp, last = pending.pop(0)
                    pv_group(op_tl, h, pt, grp)
                    if last:
                        norm_head(h, op_tl)

                for h in range(HPC):
                    qt_h = qt_s[h // 2]          # packed, full 128 rows
                    kt_h = kt_z[h]               # zero-padded stationary
                    op_tl = pop.tile([128, 512], F32, tag="po")
                    for gi, grp in enumerate(groups):
                        sp = ssp.tile([128, 1024], F32, tag="ss")
                        for (kc, lo, n, off) in grp:
                            nc.tensor.matmul(
                                sp[:, off:off + n],
                                kt_h[:, kc * 128:kc * 128 + 128],
                                qt_h[:, lo:hi0], start=True, stop=True)
                        end = grp[-1][3] + grp[-1][2]
                        pt = ppp.tile([128, 1024], F16, tag="p")
                        nc.scalar.activation(pt[:, 0:end], sp[:, 0:end],
                                             AF.Exp, scale=float(SCALE))
                        for (kc, lo, n, off) in grp:
                            if kc * 128 >= lo0:  # diagonal block
                                nc.vector.tensor_tensor(
                                    pt[:, off:off + 128],
                                    pt[:, off:off + 128], mask[:], op=MUL)
                        pending.append((op_tl, h, pt, grp,
                                        gi == len(groups) - 1))
                        if take[slot] and fillers:
                            fillers.pop()()
                        slot += 1
                        while len(pending) > 2:
                            pop_pending()
                while pending:
                    pop_pending()
                    if fillers:
                        fillers.pop()()
                while fillers:
                    fillers.pop()()
            for u in proj_units(NT4 - 1):
                u()

    nc.finalize()
    return nc


def _get_program():
    if "nc" not in _cache:
        _ensure_env()
        _cache["nc"] = _build()
    return _cache["nc"]


def kernel(x, w_attn, b_attn, w_proj, b_proj):
    x = np.asarray(x, dtype=np.float32)
    w_attn = np.asarray(w_attn, dtype=np.float32)
    b_attn = np.asarray(b_attn, dtype=np.float32)
    w_proj = np.asarray(w_proj, dtype=np.float32)
    b_proj = np.asarray(b_proj, dtype=np.float32)

    nc = _get_program()
    from concourse.bass_utils import run_bass_kernel_spmd

    mask = np.triu(np.ones((128, 128), dtype=np.float16))

    def arrange_w(w):
        # [C, QC] -> [128, NKC*QC]: row p, col c*QC+n = w[c*128+p, n]
        return np.ascontiguousarray(
            w.reshape(NKC, 128, QC).transpose(1, 0, 2).reshape(128, -1)
            .astype(np.float16))

    in_maps = []
    for c in range(NCORES):
        b = c // 4
        hg = c % 4
        q0 = hg * QC
        # x^T window-packed: [128, (t4, c, t)] with
        # value = x[b][t4*512+t, c*128+p]
        xtw = np.ascontiguousarray(
            x[b].astype(np.float16).T            # [C, T]
            .reshape(NKC, 128, NT4, 512)         # [c, p, t4, t]
            .transpose(1, 2, 0, 3)               # [p, t4, c, t]
            .reshape(128, -1))
        in_maps.append({
            "xt": xtw,
            "wq": arrange_w(w_attn[:, q0:q0 + QC]),
            "wk": arrange_w(w_attn[:, C + q0:C + q0 + QC]),
            "wv": arrange_w(w_attn[:, 2 * C + q0:2 * C + q0 + QC]),
            "bq": np.ascontiguousarray(
                b_attn[q0:q0 + QC].reshape(2, 128).T),
            "wp": np.ascontiguousarray(
                w_proj[q0:q0 + QC, :].astype(np.float16)),
            "mask": mask,
        })

    trace = bool(os.environ.get("KERNEL_TRACE"))
    res = run_bass_kernel_spmd(nc, in_maps, list(range(NCORES)), trace=trace)
    _cache["last_results"] = res

    # b_k cancels in the softmax; b_v contributes bv @ W_proj, constant
    # over t, so it joins b_proj at gather time.
    bias = b_proj + b_attn[2 * C:] @ w_proj
    out = np.empty((B, T, C), dtype=np.float32)
    for b in range(B):
        acc = res.results[4 * b]["yt"].astype(np.float32)
        for c in range(4 * b + 1, 4 * b + 4):
            acc = acc + res.results[c]["yt"].astype(np.float32)
        out[b] = acc.T + bias
    return out
